# revision 14
# baseline (speedup 1.0000x reference)
"""Trainium2 Bass kernel for nn_NexusV2 (CentroidAddressableManifold.read).

Strategy: shard by *bucket* (not token). Tokens are routed host-side to the
core owning their bucket; each bucket's 32 slot rows are loaded exactly once
from HBM (vs. the reference's per-token gather => ~8x less memory traffic).

Device layout (per core, all shapes static at trace time):
  - tokens are packed into "instances" of <=16 tokens sharing one bucket
    (buckets with >16 tokens split into several instances)
  - groups of <=8 instances => <=128 token rows x <=256 slot columns
  - per group: PE computes scores = unified_query @ K^T (token-major,
    float32r), softmax + hard-match path on DVE/ACT, val = probs @ V on PE.

Host does only routing/permutation + transposed packing of the read-only
tables; all FLOPs of the reference (norms, dots, softmax, matches, matmuls)
run on device.
"""

import math
import sys
import types

import numpy as np

N_BUCKETS = 512
SPB = 32          # slots per bucket
TAU = 0.1
P_PAD = 16        # token rows per instance
IPG = 8           # instances per (full) group
N_CORES = 8
D = 1024
KCH = 8           # D / 128 contraction chunks
NEG = -30000.0    # additive mask value

_COMPILED = {}    # plan -> (nc, names)
_HOOK_DONE = False


# ----------------------------------------------------------------- utilities

def _install_ntff_hook():
    """Synthesize antenv.axon_hooks so trace=True can NTFF-profile (optional)."""
    global _HOOK_DONE
    if _HOOK_DONE or 'antenv.axon_hooks' in sys.modules:
        _HOOK_DONE = True
        return
    try:
        import antenv
        m = types.ModuleType('antenv.axon_hooks')
        _hook = [None]
        m.set_axon_ntff_profile_hook = lambda h: _hook.__setitem__(0, h)
        m.get_axon_ntff_profile_hook = lambda: _hook[0]
        sys.modules['antenv.axon_hooks'] = m
        antenv.axon_hooks = m
        if '/root/.axon_site' not in sys.path:
            sys.path.insert(0, '/root/.axon_site')
        from trn_agent_boot.trn_boot import _ntff_profile_via_ctypes
        m.set_axon_ntff_profile_hook(
            _ntff_profile_via_ctypes('/opt/axon/libaxon_pjrt.so'))
    except Exception:
        pass
    _HOOK_DONE = True


def _routing(tids_flat):
    """Return list of instances: (bucket_id, np.array of <=16 token indices)."""
    buckets = (tids_flat.astype(np.int64)) % N_BUCKETS
    order = np.argsort(buckets, kind='stable')
    counts = np.bincount(buckets, minlength=N_BUCKETS)
    cum = np.concatenate([[0], np.cumsum(counts)])
    instances = []
    for b in range(N_BUCKETS):
        c = int(counts[b])
        if c == 0:
            continue
        toks = order[cum[b]:cum[b] + c]
        for i in range(0, c, P_PAD):
            instances.append((b, toks[i:i + P_PAD]))
    return instances


def _plan(n_inst):
    i_core = (n_inst + N_CORES - 1) // N_CORES
    ngs, r = [], i_core
    while r > 0:
        ngs.append(min(IPG, r))
        r -= min(IPG, r)
    return i_core, tuple(ngs)


def _group_geom(ng):
    """Column geometry inside a group's kv block: KCH chunks of [K^T_k|a^T_k]
    (ns+ngp cols each), then nv V blocks of D cols. ngp = anchor dim padded
    even for fp32r matmul free-size restrictions."""
    ns = SPB * ng
    nv = 1 if ns <= 128 else 2
    ngp = ng + (ng % 2)
    return ns, nv, ngp, KCH * (ns + ngp) + nv * D


def _consts():
    r = np.arange(128)
    c256 = np.arange(256)
    valid = (c256[None, :] // SPB) == (r[:, None] // P_PAD)
    winadd = np.where(valid, 0.0, NEG).astype(np.float32)
    win01 = valid.astype(np.float32)
    oh8 = (np.arange(IPG)[None, :] == (r[:, None] // P_PAD)).astype(np.float32)
    oh8t_half = (0.5 * oh8.T).astype(np.float32)
    ident = np.eye(128, dtype=np.float32)
    return winadd, win01, oh8, oh8t_half, ident


def _pack_core(insts, ngs, q_flat, tids_flat, KT, V, slot_tids, CBT):
    """Build this core's input arrays. insts: list of (bucket, toks) or None."""
    i16 = sum(ngs) * P_PAD
    qr = np.zeros((i16, D), np.float32)
    trp = np.full((i16, 1), -1.0, np.float32)
    tidb = np.full((len(ngs), 2 * 128), -2.0, np.float32)
    tok_idx = np.full(i16, -1, np.int64)

    wtot = sum(_group_geom(ng)[3] for ng in ngs)
    kv = np.zeros((128, wtot), np.float32)

    col = 0
    row = 0
    ii = 0
    for g, ng in enumerate(ngs):
        ns, nv, ngp, wg = _group_geom(ng)
        group = insts[ii:ii + ng]
        ii += ng
        # slot ids (and bucket list) for this group
        slot_ids = np.zeros(ns, np.int64)
        real_slots = np.zeros(ns, bool)
        bucks = np.zeros(ng, np.int64)
        real_inst = np.zeros(ng, bool)
        for j, inst in enumerate(group):
            if inst is None:
                continue
            b, toks = inst
            bucks[j] = b
            real_inst[j] = True
            slot_ids[j * SPB:(j + 1) * SPB] = np.arange(b * SPB, (b + 1) * SPB)
            real_slots[j * SPB:(j + 1) * SPB] = True
            tidb[g, j * SPB:(j + 1) * SPB] = slot_tids[b * SPB:(b + 1) * SPB]
            r0 = row + j * P_PAD
            nt_real = len(toks)
            qr[r0:r0 + nt_real] = q_flat[toks]
            trp[r0:r0 + nt_real, 0] = tids_flat[toks]
            tok_idx[r0:r0 + nt_real] = toks
        # ka chunks [128, KCH, ns+ngp]: per chunk K^T slots then anchors
        ktg = KT[:, slot_ids].reshape(KCH, 128, ns) * real_slots[None, None, :]
        atp = np.zeros((KCH, 128, ngp), np.float32)
        atp[:, :, 0:ng] = CBT[:, bucks].reshape(KCH, 128, ng) \
            * real_inst[None, None, :]
        ka = np.concatenate([ktg, atp], axis=2)        # [KCH, 128, ns+ngp]
        kv[:, col:col + KCH * (ns + ngp)] = \
            ka.transpose(1, 0, 2).reshape(128, KCH * (ns + ngp))
        c = col + KCH * (ns + ngp)
        # V blocks
        vb = V[slot_ids] * real_slots[:, None]          # [ns, D]
        for h in range(nv):
            nsh = min(128, ns - h * 128)
            kv[0:nsh, c:c + D] = vb[h * 128:h * 128 + nsh]
            c += D
        col += wg
        row += ng * P_PAD
    return dict(qr=qr, kv=kv, tidb=tidb, trp=trp), tok_idx


# ------------------------------------------------------------- device kernel

def _build_nc(ngs, i16):
    from concourse import bacc, mybir, tile

    F32 = mybir.dt.float32
    F32R = mybir.dt.float32r
    AL = mybir.AluOpType
    AF = mybir.ActivationFunctionType
    X = mybir.AxisListType.X

    geoms = [_group_geom(ng) for ng in ngs]
    wtot = sum(g[3] for g in geoms)
    n_groups = len(ngs)
    kcols = np.concatenate([[0], np.cumsum([g[3] for g in geoms])])
    rows = np.concatenate([[0], np.cumsum([ng * P_PAD for ng in ngs])])

    nc = bacc.Bacc(trn_type="TRN2", target_bir_lowering=False, debug=False)
    d_qr = nc.dram_tensor("qr", [i16, D], F32, kind="ExternalInput").ap()
    d_kv = nc.dram_tensor("kv", [128, wtot], F32R, kind="ExternalInput").ap()
    d_tidb = nc.dram_tensor("tidb", [n_groups, 256], F32, kind="ExternalInput").ap()
    d_trp = nc.dram_tensor("trp", [i16, 1], F32, kind="ExternalInput").ap()
    d_winadd = nc.dram_tensor("winadd", [128, 256], F32, kind="ExternalInput").ap()
    d_win01 = nc.dram_tensor("win01", [128, 256], F32, kind="ExternalInput").ap()
    d_oh8 = nc.dram_tensor("oh8", [128, IPG], F32, kind="ExternalInput").ap()
    d_oh8t = nc.dram_tensor("oh8t", [IPG, 128], F32R, kind="ExternalInput").ap()
    d_ident = nc.dram_tensor("identw", [128, 128], F32, kind="ExternalInput").ap()
    d_out = nc.dram_tensor("outp", [i16, D], F32, kind="ExternalOutput").ap()

    with tile.TileContext(nc) as tc:
        with tc.tile_pool(name="const", bufs=1) as pc, \
             tc.tile_pool(name="kvp", bufs=4) as pkv, \
             tc.tile_pool(name="io", bufs=3) as pio, \
             tc.tile_pool(name="wk", bufs=2) as pw, \
             tc.tile_pool(name="ps", bufs=1, space="PSUM") as pp, \
             tc.tile_pool(name="ps2", bufs=2, space="PSUM") as pp2:

            winadd = pc.tile([128, 256], F32)
            win01 = pc.tile([128, 256], F32)
            oh8 = pc.tile([128, IPG], F32)
            oh8t = pc.tile([IPG, 128], F32R)
            ident = pc.tile([128, 128], F32)
            nc.sync.dma_start(winadd[:], d_winadd)
            nc.sync.dma_start(win01[:], d_win01)
            nc.sync.dma_start(oh8[:], d_oh8)
            nc.sync.dma_start(oh8t[:], d_oh8t)
            nc.sync.dma_start(ident[:], d_ident)
            eps24 = pc.tile([128, 1], F32)
            nc.gpsimd.memset(eps24[:], 1e-24)

            for g, ng in enumerate(ngs):
                ns, nv, ngp, wg = geoms[g]
                nt = ng * P_PAD
                nsp = ns + ngp
                col = kcols[g]

                kv_t = pkv.tile([128, 4160], F32R, tag="kv")
                nc.sync.dma_start(kv_t[:, 0:wg], d_kv[:, col:col + wg])
                ka = kv_t[:, 0:KCH * nsp].rearrange("p (k s) -> p k s", k=KCH)
                q_t = pio.tile([128, D], F32, tag="q")
                nc.scalar.dma_start(q_t[0:nt, :], d_qr[rows[g]:rows[g] + nt, :])
                tidb_row = pio.tile([1, 256], F32, tag="tidbrow")
                nc.scalar.dma_start(tidb_row[:, 0:ns], d_tidb[g:g + 1, 0:ns])
                tidb_t = pio.tile([128, 256], F32, tag="tidb")
                nc.gpsimd.partition_broadcast(tidb_t[0:nt, 0:ns],
                                              tidb_row[:, 0:ns])
                tr_t = pio.tile([128, 1], F32, tag="tr")
                nc.scalar.dma_start(tr_t[0:nt, :], d_trp[rows[g]:rows[g] + nt, :])

                # --- normalize queries: qs = 0.5 * q / ||q||
                sq = pw.tile([128, D], F32, tag="sq")
                ssq = pw.tile([128, 1], F32, tag="ssq")
                nc.scalar.activation(sq[0:nt, :], q_t[0:nt, :], AF.Square,
                                     accum_out=ssq[0:nt, :])
                n2 = pw.tile([128, 1], F32, tag="n2")
                nc.scalar.activation(n2[0:nt, :], ssq[0:nt, :], AF.Sqrt,
                                     scale=4.0, bias=eps24[0:nt, :])
                rq2 = pw.tile([128, 1], F32, tag="rq2")
                nc.vector.reciprocal(rq2[0:nt, :], n2[0:nt, :])
                qs = pw.tile([128, D], F32, tag="qs")
                nc.gpsimd.tensor_scalar(out=qs[0:nt, :], in0=q_t[0:nt, :],
                                        scalar1=rq2[0:nt, :], scalar2=None,
                                        op0=AL.mult)

                # --- transpose scaled queries -> qt [128d, KCH, nt] (f32r)
                qt = pw.tile([128, KCH, 128], F32R, tag="qt")
                for hb in range(2):
                    qth = pp2.tile([128, 512], F32, tag="qth")
                    for k in range(4):
                        kk = hb * 4 + k
                        nc.tensor.transpose(
                            qth[:, k * 128:k * 128 + nt],
                            qs[0:nt, kk * 128:(kk + 1) * 128],
                            ident[0:nt, 0:nt])
                    nc.vector.tensor_copy(
                        qt[:, hb * 4:(hb + 1) * 4, 0:nt],
                        qth[:].rearrange("p (k t) -> p k t", k=4)[:, :, 0:nt])

                # --- anchor-dot table a0t = a.K  [ngp, ns]
                a0t_ps = pp.tile([IPG, 256], F32, tag="a0t")
                for k in range(KCH):
                    nc.tensor.matmul(a0t_ps[0:ngp, 0:ns], ka[:, k, ns:nsp],
                                     ka[:, k, 0:ns], start=(k == 0),
                                     stop=(k == KCH - 1))
                a0t = pw.tile([IPG, 256], F32R, tag="a0tsb")
                nc.vector.tensor_copy(a0t[0:ngp, 0:ns], a0t_ps[0:ngp, 0:ns])

                # --- scores(+qa cols)+blend: [qn'.K | qn'.a] + 0.5*a.K
                sc_ps = pp.tile([128, 264], F32, tag="sc")
                for k in range(KCH):
                    nc.tensor.matmul(sc_ps[0:nt, 0:nsp], qt[:, k, 0:nt],
                                     ka[:, k, 0:nsp], start=(k == 0),
                                     stop=False)
                nc.tensor.matmul(sc_ps[0:nt, 0:ns], oh8t[0:ngp, 0:nt],
                                 a0t[0:ngp, 0:ns], start=False, stop=True)

                # --- rw = 1/|W| from qa cols; rw10 = rw/tau
                qasc = pw.tile([128, IPG], F32, tag="qasc")
                qa1 = pw.tile([128, 1], F32, tag="qa1")
                nc.vector.tensor_tensor(out=qasc[0:nt, 0:ngp],
                                        in0=sc_ps[0:nt, ns:nsp],
                                        in1=oh8[0:nt, 0:ngp], op=AL.mult)
                nc.vector.reduce_sum(qa1[0:nt, :], qasc[0:nt, 0:ngp], axis=X)
                w2 = pw.tile([128, 1], F32, tag="w2")
                nc.vector.tensor_scalar(out=w2[0:nt, :], in0=qa1[0:nt, :],
                                        scalar1=0.5, scalar2=None, op0=AL.add)
                wn = pw.tile([128, 1], F32, tag="wn")
                nc.scalar.activation(wn[0:nt, :], w2[0:nt, :], AF.Sqrt)
                rw = pw.tile([128, 1], F32, tag="rw")
                nc.vector.reciprocal(rw[0:nt, :], wn[0:nt, :])
                rw10 = pw.tile([128, 1], F32, tag="rw10")
                nc.vector.tensor_scalar(out=rw10[0:nt, :], in0=rw[0:nt, :],
                                        scalar1=1.0 / TAU, scalar2=None,
                                        op0=AL.mult)

                # --- masked scores, softmax with rw/tau in Exp scale
                sc = pw.tile([128, 256], F32, tag="scsb")
                nc.vector.tensor_tensor(out=sc[0:nt, 0:ns],
                                        in0=sc_ps[0:nt, 0:ns],
                                        in1=winadd[0:nt, 0:ns], op=AL.add)
                negmax = pw.tile([128, 1], F32, tag="negmax")
                nc.vector.reduce_max(negmax[0:nt, :], sc[0:nt, 0:ns], axis=X,
                                     negate=True)
                ebias = pw.tile([128, 1], F32, tag="ebias")
                nc.vector.tensor_tensor(out=ebias[0:nt, :], in0=negmax[0:nt, :],
                                        in1=rw10[0:nt, :], op=AL.mult)
                ex = pw.tile([128, 256], F32, tag="ex")
                esum = pw.tile([128, 1], F32, tag="esum")
                nc.scalar.activation(ex[0:nt, 0:ns], sc[0:nt, 0:ns], AF.Exp,
                                     bias=ebias[0:nt, :], scale=rw10[0:nt, :],
                                     accum_out=esum[0:nt, :])
                rsum = pw.tile([128, 1], F32, tag="rsum")
                nc.vector.reciprocal(rsum[0:nt, :], esum[0:nt, :])

                # --- hard match path
                match = pw.tile([128, 256], F32, tag="match")
                msum = pw.tile([128, 1], F32, tag="msum")
                nc.vector.scalar_tensor_tensor(
                    out=match[0:nt, 0:ns], in0=tidb_t[0:nt, 0:ns],
                    scalar=tr_t[0:nt, :], in1=win01[0:nt, 0:ns],
                    op0=AL.is_equal, op1=AL.mult, accum_out=msum[0:nt, :])
                mden = pw.tile([128, 1], F32, tag="mden")
                nc.vector.tensor_scalar(out=mden[0:nt, :], in0=msum[0:nt, :],
                                        scalar1=1e-9, scalar2=None, op0=AL.add)
                mrec = pw.tile([128, 1], F32, tag="mrec")
                nc.vector.reciprocal(mrec[0:nt, :], mden[0:nt, :])
                nohas = pw.tile([128, 1], F32, tag="nohas")
                nc.vector.tensor_scalar(out=nohas[0:nt, :], in0=msum[0:nt, :],
                                        scalar1=0.0, scalar2=None, op0=AL.is_le)
                hard = pw.tile([128, 256], F32, tag="hard")
                nc.gpsimd.tensor_scalar(out=hard[0:nt, 0:ns],
                                        in0=match[0:nt, 0:ns],
                                        scalar1=mrec[0:nt, :], scalar2=None,
                                        op0=AL.mult)
                rs_nh = pw.tile([128, 1], F32, tag="rs_nh")
                nc.vector.tensor_tensor(out=rs_nh[0:nt, :], in0=rsum[0:nt, :],
                                        in1=nohas[0:nt, :], op=AL.mult)
                probs = pw.tile([128, 256], F32, tag="probs")
                nc.vector.scalar_tensor_tensor(
                    out=probs[0:nt, 0:ns], in0=ex[0:nt, 0:ns],
                    scalar=rs_nh[0:nt, :], in1=hard[0:nt, 0:ns],
                    op0=AL.mult, op1=AL.add)

                # --- probs^T, then val = probs @ V
                pt_ps = pp.tile([128, 264], F32, tag="pt")
                for h in range(nv):
                    nsh = min(128, ns - h * 128)
                    nc.tensor.transpose(pt_ps[0:nsh, h * 128:h * 128 + nt],
                                        probs[0:nt, h * 128:h * 128 + nsh],
                                        ident[0:nt, 0:nt])
                pt = pw.tile([128, 2, 128], F32R, tag="ptsb")
                for h in range(nv):
                    nsh = min(128, ns - h * 128)
                    nc.vector.tensor_copy(pt[0:nsh, h, 0:nt],
                                          pt_ps[0:nsh, h * 128:h * 128 + nt])
                pv = pp.tile([128, D], F32, tag="pv")
                for j in range(2):
                    for h in range(nv):
                        nsh = min(128, ns - h * 128)
                        nc.tensor.matmul(
                            pv[0:nt, j * 512:(j + 1) * 512],
                            pt[0:nsh, h, 0:nt],
                            kv_t[0:nsh, KCH * nsp + h * D + j * 512:
                                 KCH * nsp + h * D + (j + 1) * 512],
                            start=(h == 0), stop=(h == nv - 1))
                out_sb = pw.tile([128, D], F32, tag="out_sb")
                nc.vector.tensor_copy(out_sb[0:nt, :], pv[0:nt, :])
                nc.scalar.dma_start(d_out[rows[g]:rows[g] + nt, :],
                                    out_sb[0:nt, :])
    nc.compile()
    return nc


# ------------------------------------------------------------------ emulator

def _emulate_core(ins, ngs):
    """Numpy emulation of the device kernel (fp32), for validation."""
    qr, kv, tidb, trp = ins["qr"], ins["kv"], ins["tidb"], ins["trp"]
    i16 = qr.shape[0]
    out = np.zeros((i16, D), np.float32)
    winadd, win01, oh8, oh8t, _ = _consts()
    col = row = 0
    for g, ng in enumerate(ngs):
        ns, nv, ngp, wg = _group_geom(ng)
        nt = ng * P_PAD
        ka = kv[:, col:col + KCH * (ns + ngp)].reshape(128, KCH, ns + ngp)
        ktg = ka[:, :, 0:ns]
        atp = ka[:, :, ns:ns + ng]
        voff = col + KCH * (ns + ngp)
        vb = np.zeros((ns, D), np.float32)
        for h in range(nv):
            nsh = min(128, ns - h * 128)
            vb[h * 128:h * 128 + nsh] = kv[0:nsh, voff + h * D:voff + (h + 1) * D]

        q = qr[row:row + nt]
        ssq = (q * q).sum(-1, keepdims=True)
        rq2 = 1.0 / np.sqrt(4 * ssq + 1e-24)
        qn = q * rq2                                   # 0.5 * normalized
        KT = ktg.transpose(1, 0, 2).reshape(D, ns)     # [D, ns]
        AT = atp.transpose(1, 0, 2).reshape(D, ng)     # [D, ng]
        a0t = AT.T @ KT                                # [ng, ns]
        sc_ps = qn @ KT + (0.5 * oh8[0:nt, 0:ng]) @ a0t
        qa1 = ((qn @ AT) * oh8[0:nt, 0:ng]).sum(-1, keepdims=True)
        rw = 1.0 / np.sqrt(qa1 + 0.5)
        sc = sc_ps * rw + winadd[0:nt, 0:ns]
        m = sc.max(-1, keepdims=True)
        ex = np.exp((sc - m) / TAU)
        esum = ex.sum(-1, keepdims=True)
        match = (tidb[g, 0:ns][None, :] == trp[row:row + nt]) * win01[0:nt, 0:ns]
        msum = match.sum(-1, keepdims=True)
        nohas = (msum <= 0).astype(np.float32)
        hard = match / (msum + 1e-9)
        probs = ex * (nohas / esum) + hard
        out[row:row + nt] = probs.astype(np.float32) @ vb
        col += wg
        row += nt
    return out


# -------------------------------------------------------------------- kernel

def kernel(query_emb, tids, slot_keys, slot_values, slot_tids,
           centroid_codebook, _emulate=False, _trace=False):
    B, T, _ = query_emb.shape
    BT = B * T
    q_flat = np.ascontiguousarray(query_emb.reshape(BT, D), np.float32)
    tids_flat = np.asarray(tids).reshape(BT)
    st = np.asarray(slot_tids).astype(np.float32)
    KT = np.ascontiguousarray(np.asarray(slot_keys, np.float32).T)     # [D, S]
    V = np.asarray(slot_values, np.float32)
    CBT = np.ascontiguousarray(np.asarray(centroid_codebook, np.float32).T)

    instances = _routing(tids_flat)
    i_core, ngs = _plan(len(instances))
    padded = instances + [None] * (i_core * N_CORES - len(instances))
    i16 = i_core * P_PAD

    winadd, win01, oh8, oh8t, ident = _consts()
    in_maps, tok_idxs = [], []
    for c in range(N_CORES):
        ins, tok_idx = _pack_core(padded[c * i_core:(c + 1) * i_core], ngs,
                                  q_flat, tids_flat, KT, V, st, CBT)
        ins.update(winadd=winadd, win01=win01, oh8=oh8, oh8t=oh8t,
                   identw=ident)
        in_maps.append(ins)
        tok_idxs.append(tok_idx)

    out_flat = np.zeros((BT, D), np.float32)
    if _emulate:
        for c in range(N_CORES):
            o = _emulate_core(in_maps[c], ngs)
            valid = tok_idxs[c] >= 0
            out_flat[tok_idxs[c][valid]] = o[valid]
        return out_flat.reshape(B, T, D).astype(np.float32)

    _install_ntff_hook()
    from concourse import bass_utils
    key = (ngs, i16)
    if key not in _COMPILED:
        _COMPILED[key] = _build_nc(ngs, i16)
    nc = _COMPILED[key]
    res = bass_utils.run_bass_kernel_spmd(
        nc, in_maps, core_ids=list(range(N_CORES)), trace=_trace)
    for c in range(N_CORES):
        o = res.results[c]["outp"]
        valid = tok_idxs[c] >= 0
        out_flat[tok_idxs[c][valid]] = o[valid]
    out = out_flat.reshape(B, T, D).astype(np.float32)
    if _trace:
        kernel._last_exec_time_ns = res.exec_time_ns
        kernel._last_results = res
    return out
